# revision 1
# baseline (speedup 1.0000x reference)
"""BitLinear (ternary weight quantization + linear) on 8 Trainium2 NeuronCores.

Math: out = (x @ w_q.T + b) * LAYER_SCALE, where
  beta = max(mean(|W|), eps)           (global scalar over the full W)
  w_q  = clip(round(W / beta), -1, 1) * beta   (ternary: beta * {-1, 0, +1})

Device strategy (per the column-parallel sharding hint, plus data-parallel):
  8 cores = 2 batch-shards (tokens) x 4 feature-shards (out_features).
  Each core: quantize its fp32 W shard to ternary {-1,0,+1} in bf16
  (exact), then a bf16 matmul accumulating fp32 in PSUM, then a fused
  scale+bias drain on the Scalar engine. Ternary weights are exact in
  bf16, so the only precision loss is the bf16 rounding of x (~1.6e-3
  relative), which is identical whether x is rounded on host or device.

Numerical care: round(W/beta) decisions near |W/beta| = 0.5 flip with the
last ulp of beta. beta is therefore computed on host with jax-CPU exactly
as the reference does, and the round-half-to-even decision is lowered to
an exact fp32 threshold compare |W| > c where c is the largest float32
with fl32(c/beta) <= 0.5 (verified bit-identical to the reference
quantization). On device the quantization is then two compares + a
subtract per element — no rounding-mode hazards.
"""

import math
from functools import lru_cache

import ml_dtypes
import numpy as np

import concourse.bass as bass
import concourse.mybir as mybir
import concourse.tile as tile
from concourse import bacc
from concourse.bass import ts
from concourse.bass_utils import run_bass_kernel_spmd

P = 128
IN_FEATURES = 2048
OUT_FEATURES = 8192
N_TOKENS = 8192  # 4 * 2048
EPS = 1e-8
LAYER_SCALE = np.float32(1.0 / math.sqrt(IN_FEATURES))

S_WAYS = 2  # data-parallel over tokens
Q_WAYS = 4  # tensor-parallel over out_features
N_CORES = S_WAYS * Q_WAYS

F32 = mybir.dt.float32
BF16 = mybir.dt.bfloat16


@lru_cache(maxsize=4)
def build_nc(KI: int, OC: int, TC: int, TB: int = 512):
    """Per-core bass program.

    Inputs (per core):
      xt     [KI, TC] bf16: x^T shard (in_features x tokens), host-rounded
      wt     [KI, OC] f32 : W^T shard (in_features x out_features)
      bvec   [OC]     f32 : bias shard, host-reordered (see below)
      consts [P, 3]   f32 : [c, -c, beta*LAYER_SCALE] per partition
    Output:
      out  [OC, TC] f32 : (x @ w_q.T)^T shard, scaled and biased
    """
    assert KI % P == 0 and OC % P == 0 and TC % TB == 0
    K_TILES = KI // P
    M_TILES = OC // P
    T_BLOCKS = TC // TB
    KG = min(4, K_TILES)  # k-tiles per W staging DMA
    N_KG = K_TILES // KG
    XKG = min(4, K_TILES)  # k-tiles per x group
    N_XKG = K_TILES // XKG
    MG = min(4, M_TILES)  # m-tiles per output DMA
    assert K_TILES % KG == 0 and K_TILES % XKG == 0 and M_TILES % MG == 0

    nc = bacc.Bacc(None, target_bir_lowering=False, name="bitlinear")

    xt = nc.dram_tensor("xt", [KI, TC], BF16, kind="ExternalInput")
    wt = nc.dram_tensor("wt", [KI, OC], F32, kind="ExternalInput")
    bvec = nc.dram_tensor("bvec", [OC], F32, kind="ExternalInput")
    # columns: [cut, -cut, beta*LAYER_SCALE], identical across partitions
    consts = nc.dram_tensor("consts", [P, 3], F32, kind="ExternalInput")
    out = nc.dram_tensor("out", [OC, TC], F32, kind="ExternalOutput")

    xt_r = xt[:].rearrange("(g p) t -> p g t", p=P)  # [P, K_TILES, TC]
    wt_r = wt[:].rearrange("(g p) o -> p g o", p=P)  # [P, K_TILES, OC]
    out_r = out[:].rearrange("(g p) t -> p g t", p=P)  # [P, M_TILES, TC]

    with tile.TileContext(nc) as tc:
        with (
            tc.tile_pool(name="const", bufs=1) as cpool,
            tc.tile_pool(name="wq", bufs=1) as wqpool,
            tc.tile_pool(name="xb", bufs=3) as xbpool,
            tc.tile_pool(name="ot", bufs=2) as opool,
            tc.tile_pool(name="ps", bufs=6, space="PSUM") as pspool,
        ):
            # --- constants (one small DMA so the W stream isn't blocked) ---
            cst = cpool.tile([P, 3], F32)
            bt = cpool.tile([P, M_TILES], F32)
            bs = cpool.tile([P, M_TILES], F32)
            nc.sync.dma_start(cst[:], consts[:])
            cut_t = cst[:, 0:1]
            ncut_t = cst[:, 1:2]
            scl_t = cst[:, 2:3]
            # bvec comes host-reordered so [p, m] = b[m*128 + p] is a
            # contiguous per-partition load; the scale runs on the (idle)
            # GpSimd engine to stay off the DVE/ACT critical paths.
            nc.sync.dma_start(bt[:], bvec[:].rearrange("(p m) -> p m", p=P))
            nc.gpsimd.tensor_scalar_mul(bs[:], bt[:], float(LAYER_SCALE))

            # x arrives pre-rounded to bf16: blocks DMA straight into the
            # matmul operand tiles, no cast stage at all.
            def load_x_group(tb, g):
                xg = xbpool.tile([P, XKG, TB], BF16, tag=f"xbg{g}", name=f"xbg{g}")
                nc.sync.dma_start(xg[:], xt_r[:, ts(g, XKG), ts(tb, TB)])
                return xg

            def load_x_block(tb):
                return [load_x_group(tb, g) for g in range(N_XKG)]

            # --- quantize W shard to ternary bf16, chunk-major so the PE's
            # m-ascending consumption follows production order. The first W
            # column's staging DMAs interleave with tb0's x tiles so both
            # streams start immediately. ---
            CHUNK = min(512, OC)
            N_CHUNKS = OC // CHUNK
            M_PER_CHUNK = CHUNK // P
            # wq[k][c] : [P, CHUNK] bf16 with {-1, 0, +1}
            wq = [[None] * N_CHUNKS for _ in range(K_TILES)]
            with (
                tc.tile_pool(name="wstage", bufs=4) as wspool,
                tc.tile_pool(name="qtmp", bufs=6) as qpool,
            ):

                def load_w_group(c, kg):
                    wst = wspool.tile([P, KG, CHUNK], F32, tag="wst")
                    nc.sync.dma_start(wst[:], wt_r[:, ts(kg, KG), ts(c, CHUNK)])
                    return wst

                def quant_one(c, k, wst_slice):
                    neg = qpool.tile([P, CHUNK], F32, tag="neg", name="neg")
                    wq_kc = wqpool.tile(
                        [P, CHUNK], BF16, tag=f"wq{k}_{c}", name=f"wq{k}_{c}"
                    )
                    nc.vector.tensor_scalar(
                        neg[:],
                        wst_slice,
                        ncut_t[:, 0:1],
                        None,
                        mybir.AluOpType.is_lt,
                    )
                    # wq = (W > c) - (W < -c)
                    nc.vector.scalar_tensor_tensor(
                        wq_kc[:],
                        wst_slice,
                        cut_t[:, 0:1],
                        neg[:],
                        mybir.AluOpType.is_gt,
                        mybir.AluOpType.subtract,
                    )
                    wq[k][c] = wq_kc

                def quant_group(c, kg, wst):
                    for kk in range(KG):
                        quant_one(c, kg * KG + kk, wst[:, kk, :])

                # column 0 W DMAs interleaved with tb0's (and tb1's) x
                # blocks: the alternating schedule consumes each weight
                # chunk twice, so both x blocks are needed from the start.
                xb0 = []
                xb1 = [] if T_BLOCKS >= 2 else None
                w0 = []
                emitted = set()
                for kg in range(N_KG):
                    w0.append(load_w_group(0, kg))
                    xg_idx = kg * N_XKG // N_KG
                    if xg_idx not in emitted:
                        emitted.add(xg_idx)
                        xb0.append(load_x_group(0, xg_idx))
                        if xb1 is not None:
                            xb1.append(load_x_group(1, xg_idx))
                for kg in range(N_KG):
                    quant_group(0, kg, w0[kg])
                for c in range(1, N_CHUNKS):
                    tiles = [load_w_group(c, kg) for kg in range(N_KG)]
                    for kg in range(N_KG):
                        quant_group(c, kg, tiles[kg])

            # --- main loop: matmul + fused drain, batched output DMA ---
            ot_cur = {}  # mg -> (tile, tb)

            def flush_ot(mg):
                if mg in ot_cur:
                    t, tb_prev = ot_cur.pop(mg)
                    nc.sync.dma_start(
                        out_r[:, ts(mg, MG), ts(tb_prev, TB)], t[:]
                    )

            def mm_tile(tb, m, xb, flush_each=False):
                c, mi = divmod(m, M_PER_CHUNK)
                ps = pspool.tile([P, TB], F32, tag="ps")
                for k in range(K_TILES):
                    g, kk = divmod(k, XKG)
                    nc.tensor.matmul(
                        ps[:],
                        wq[k][c][:, ts(mi, P)],
                        xb[g][:, kk, :],
                        start=(k == 0),
                        stop=(k == K_TILES - 1),
                    )
                mg, mgi = divmod(m, MG)
                if mgi == 0:
                    flush_ot(mg)
                    ot_tile = opool.tile(
                        [P, MG, TB], F32, tag=f"ot{mg % 2}", name=f"ot{mg % 2}"
                    )
                    ot_cur[mg] = (ot_tile, tb)
                ot, _ = ot_cur[mg]
                # ot = psum * (beta * LAYER_SCALE) + b * LAYER_SCALE
                nc.scalar.activation(
                    ot[:, mgi, :],
                    ps[:],
                    mybir.ActivationFunctionType.Identity,
                    bias=bs[:, m : m + 1],
                    scale=scl_t[:, 0:1],
                )
                if flush_each:
                    # kernel tail: don't batch the store behind later drains
                    nc.sync.dma_start(out_r[:, m, ts(tb, TB)], ot[:, mgi, :])
                    if mgi == MG - 1:
                        ot_cur.pop(mg)
                elif mgi == MG - 1:
                    flush_ot(mg)

            if xb1 is not None and N_CHUNKS >= 2:
                # Software-pipeline tb0/tb1: alternate weight chunks between
                # the two blocks so the PE's per-chunk demand rate is half
                # the quantization production rate — it never catches up.
                for c in range(N_CHUNKS):
                    for m in range(c * M_PER_CHUNK, (c + 1) * M_PER_CHUNK):
                        mm_tile(0, m, xb0)
                    for m in range(c * M_PER_CHUNK, (c + 1) * M_PER_CHUNK):
                        mm_tile(1, m, xb1)
                done = 2
            elif xb1 is not None:
                for m in range(M_TILES):
                    mm_tile(0, m, xb0)
                for m in range(M_TILES):
                    mm_tile(1, m, xb1)
                done = 2
            else:
                for m in range(M_TILES):
                    mm_tile(0, m, xb0)
                done = 1

            for tb in range(done, T_BLOCKS):
                xb = load_x_block(tb)
                for m in range(M_TILES):
                    last_group = tb == T_BLOCKS - 1 and m >= M_TILES - MG
                    mm_tile(tb, m, xb, flush_each=last_group)
            for mg in list(ot_cur):
                flush_ot(mg)

    nc.compile()
    return nc


def _host_beta_cut(W: np.ndarray):
    """beta exactly as the (jax) reference computes it, plus the exact fp32
    threshold c reproducing round-half-to-even of W/beta near 0.5."""
    try:
        import jax
        import jax.numpy as jnp

        cpu = jax.local_devices(backend="cpu")[0]
        with jax.default_device(cpu):
            beta = np.float32(jnp.maximum(jnp.mean(jnp.abs(jnp.asarray(W))), EPS))
    except Exception:
        beta = np.float32(max(np.abs(W).astype(np.float64).mean(), EPS))

    v = np.float32(0.5) * beta  # exact (power-of-two scale)
    assert np.float32(v / beta) <= np.float32(0.5)
    while True:
        nv = np.nextafter(v, np.float32(np.inf))
        if np.float32(nv / beta) <= np.float32(0.5):
            v = nv
        else:
            break
    return beta, v


def kernel(x: np.ndarray, W: np.ndarray, b: np.ndarray) -> np.ndarray:
    out, _ = _run(x, W, b)
    return out


def _run(x, W, b, **spmd_kwargs):
    x = np.ascontiguousarray(np.asarray(x, dtype=np.float32))
    W = np.ascontiguousarray(np.asarray(W, dtype=np.float32))
    b = np.ascontiguousarray(np.asarray(b, dtype=np.float32))

    B, T, KI = x.shape
    OC_full, KI2 = W.shape
    assert KI == KI2 == IN_FEATURES and OC_full == OUT_FEATURES
    NT = B * T
    assert NT == N_TOKENS

    TC = NT // S_WAYS  # tokens per core
    OC = OUT_FEATURES // Q_WAYS  # out features per core

    beta, c = _host_beta_cut(W)
    S = np.float32(beta * LAYER_SCALE)
    consts_a = np.ascontiguousarray(
        np.broadcast_to(
            np.array([c, np.float32(-c), S], dtype=np.float32), (P, 3)
        )
    )

    xf = x.reshape(NT, KI)
    xt_s = [
        np.ascontiguousarray(xf[s * TC : (s + 1) * TC, :].T).astype(
            ml_dtypes.bfloat16
        )
        for s in range(S_WAYS)
    ]
    wt_q = [
        np.ascontiguousarray(W[q * OC : (q + 1) * OC, :].T) for q in range(Q_WAYS)
    ]
    # device expects bvec[p * M_TILES + m] = b_shard[m * 128 + p]
    m_tiles = OC // P
    b_q = [
        np.ascontiguousarray(
            b[q * OC : (q + 1) * OC].reshape(m_tiles, P).T.ravel()
        )
        for q in range(Q_WAYS)
    ]

    in_maps = []
    for s in range(S_WAYS):
        for q in range(Q_WAYS):
            in_maps.append(
                {
                    "xt": xt_s[s],
                    "wt": wt_q[q],
                    "bvec": b_q[q],
                    "consts": consts_a,
                }
            )

    nc = build_nc(KI, OC, TC)
    res = run_bass_kernel_spmd(nc, in_maps, core_ids=list(range(N_CORES)), **spmd_kwargs)

    out_full = np.empty((NT, OUT_FEATURES), dtype=np.float32)
    for s in range(S_WAYS):
        for q in range(Q_WAYS):
            piece = res.results[s * Q_WAYS + q]["out"]  # [OC, TC]
            out_full[s * TC : (s + 1) * TC, q * OC : (q + 1) * OC] = piece.T
    return out_full.reshape(B, T, OUT_FEATURES), res



# revision 2
# speedup vs baseline: 2.4487x; 2.4487x over previous
"""BitLinear (ternary weight quantization + linear) on 8 Trainium2 NeuronCores.

Math: out = (x @ w_q.T + b) * LAYER_SCALE, where
  beta = max(mean(|W|), eps)           (global scalar over the full W)
  w_q  = clip(round(W / beta), -1, 1) * beta   (ternary: beta * {-1, 0, +1})

Device strategy (column-parallel + data-parallel as the baseline):
  8 cores = 2 batch-shards (tokens) x 4 feature-shards (out_features).

All-fp8 DoubleRow contraction. Every matmul is an e4m3 DoubleRow MM
(K=256 slots per instruction, issued back-to-back at the same ~216ns rate
as a K=128 bf16 MM — measured; mixing bf16 and fp8 MMs instead slows the
whole PE stream by exactly 1.2x, so the kernel stays dtype-pure).

Precision plan (slot assignment per output tile, 13 MMs x 256 slots):
  - k-subtiles 0..C_COR-1 ("corrected"): one MM per subtile whose two slots
    carry (hi, lo) = (e4m3(x), e4m3(x - hi)) against the SAME ternary weight
    in both slots. hi+lo reconstructs x to ~7 significant bits: error
    contribution ~7e-4.
  - k-subtiles C_COR..15: pairs of subtiles share one MM (slots = the two
    subtiles' e4m3(x)): full 2x rate, e4m3 rounding error only.
  With C_COR=10: 13 MMs/tile (vs 16 bf16 MMs in the baseline), measured
  end-to-end relative L2 error 1.63e-2 (gate 2e-2).

Ternary weights are exact in e4m3. beta and the exact |W| > c ternary
threshold are computed on host exactly as the baseline (bit-identical
quantization decisions).
"""

import math
from functools import lru_cache

import ml_dtypes
import numpy as np

import concourse.bass as bass
import concourse.mybir as mybir
import concourse.tile as tile
from concourse import bacc
from concourse.bass import ts
from concourse.bass_utils import run_bass_kernel_spmd

P = 128
IN_FEATURES = 2048
OUT_FEATURES = 8192
N_TOKENS = 8192  # 4 * 2048
EPS = 1e-8
LAYER_SCALE = np.float32(1.0 / math.sqrt(IN_FEATURES))

S_WAYS = 2  # data-parallel over tokens
Q_WAYS = 4  # tensor-parallel over out_features
N_CORES = S_WAYS * Q_WAYS

K_TILES = IN_FEATURES // P       # 16 k-subtiles
C_COR = 10                       # corrected k-subtiles (hi+lo pairs)
N_FAST = K_TILES - C_COR         # subtiles at plain e4m3 (paired 2-per-MM)
assert N_FAST % 2 == 0
N_MM = C_COR + N_FAST // 2       # DoubleRow MMs per output tile
KROWS = 2 * N_MM * P             # rows of the packed x input

F32 = mybir.dt.float32
F8E4 = mybir.dt.float8e4
DR = mybir.MatmulPerfMode.DoubleRow


@lru_cache(maxsize=4)
def build_nc(KI: int, OC: int, TC: int, TB: int = 512):
    """Per-core bass program.

    Inputs (per core; xp/wt are host-relaid so every DMA is one contiguous
    descriptor per partition — DMA issue time, not bandwidth, gated startup):
      xp     [P, T_BLOCKS, N_MM, 2, TB] f8e4: packed x slots; [p, tb, g, s, t]
             = slot s of MM g (hi/lo for g<C_COR, subtile-pair hi otherwise)
      wt     [P, N_CHUNKS, N_KG, KG, CHUNK] f32: W^T shard, chunk-major
             per-partition-contiguous staging groups
      bvec   [OC]     f32 : bias shard, host-reordered
      consts [P, 3]   f32 : [c, -c, beta*LAYER_SCALE] per partition
    Output:
      out  [OC, TC] f32 : (x @ w_q.T)^T shard, scaled and biased
    """
    assert KI % P == 0 and OC % P == 0 and TC % TB == 0
    assert KI // P == K_TILES
    M_TILES = OC // P
    T_BLOCKS = TC // TB
    KG = 4                     # k-tiles per W staging DMA
    N_KG = K_TILES // KG
    MG = min(4, M_TILES)       # m-tiles per output DMA
    assert K_TILES % KG == 0 and M_TILES % MG == 0
    CHUNK = min(512, OC)
    N_CHUNKS = OC // CHUNK

    nc = bacc.Bacc(None, target_bir_lowering=False, name="bitlinear")

    xp = nc.dram_tensor("xp", [P, T_BLOCKS, N_MM, 2, TB], F8E4,
                        kind="ExternalInput")
    wt = nc.dram_tensor("wt", [P, N_CHUNKS, N_KG, KG, CHUNK], F32,
                        kind="ExternalInput")
    bvec = nc.dram_tensor("bvec", [OC], F32, kind="ExternalInput")
    consts = nc.dram_tensor("consts", [P, 3], F32, kind="ExternalInput")
    out = nc.dram_tensor("out", [OC, TC], F32, kind="ExternalOutput")

    out_r = out[:].rearrange("(g p) t -> p g t", p=P)         # [P, M_TILES, TC]

    with tile.TileContext(nc) as tc:
        with (
            tc.tile_pool(name="const", bufs=1) as cpool,
            tc.tile_pool(name="wq", bufs=1) as wqpool,
            tc.tile_pool(name="xb", bufs=3) as xbpool,
            tc.tile_pool(name="ot", bufs=3) as opool,
            tc.tile_pool(name="ps", bufs=8, space="PSUM") as pspool,
        ):
            # --- constants ---
            cst = cpool.tile([P, 3], F32)
            bt = cpool.tile([P, M_TILES], F32)
            bs = cpool.tile([P, M_TILES], F32)
            nc.sync.dma_start(cst[:], consts[:])
            cut_t = cst[:, 0:1]
            ncut_t = cst[:, 1:2]
            scl_t = cst[:, 2:3]
            nc.sync.dma_start(bt[:], bvec[:].rearrange("(p m) -> p m", p=P))
            nc.gpsimd.tensor_scalar_mul(bs[:], bt[:], float(LAYER_SCALE))

            def load_x_block(tb):
                xt = xbpool.tile([P, N_MM, 2, TB], F8E4, tag="xpb", name="xpb")
                nc.sync.dma_start(xt[:], xp[:, tb, :, :, :])
                return xt

            # --- quantize W shard into e4m3 DoubleRow pair tiles.
            # wq8[g][c]: [P, 2, CHUNK]; g<C_COR: both slots = subtile g's
            # ternary weights (slot1 copied); g>=C_COR: slot s = subtile
            # C_COR + 2*(g-C_COR) + s. Chunk-major production order. ---
            M_PER_CHUNK = CHUNK // P
            wq8 = [[None] * N_CHUNKS for _ in range(N_MM)]
            with (
                tc.tile_pool(name="wstage", bufs=4) as wspool,
                tc.tile_pool(name="qtmp", bufs=6) as qpool,
            ):

                def load_w_group(c, kg):
                    wst = wspool.tile([P, KG, CHUNK], F32, tag="wst")
                    nc.sync.dma_start(wst[:], wt[:, c, kg, :, :])
                    return wst

                def quant_one(c, k, wst_slice):
                    if k < C_COR:
                        g, s = k, 0
                    else:
                        g = C_COR + (k - C_COR) // 2
                        s = (k - C_COR) % 2
                    if s == 0 and wq8[g][c] is None:
                        wq8[g][c] = wqpool.tile(
                            [P, 2, CHUNK], F8E4, tag=f"w8{g}_{c}",
                            name=f"w8{g}_{c}"
                        )
                    dst = wq8[g][c][:, s, :]
                    neg = qpool.tile([P, CHUNK], F32, tag="neg", name="neg")
                    nc.vector.tensor_scalar(
                        neg[:],
                        wst_slice,
                        ncut_t[:, 0:1],
                        None,
                        mybir.AluOpType.is_lt,
                    )
                    # wq = (W > c) - (W < -c)
                    nc.vector.scalar_tensor_tensor(
                        dst,
                        wst_slice,
                        cut_t[:, 0:1],
                        neg[:],
                        mybir.AluOpType.is_gt,
                        mybir.AluOpType.subtract,
                    )
                    if k < C_COR:
                        # duplicate ternary weights into slot 1 (same k for
                        # both hi and lo x slots) — on the Scalar engine, to
                        # keep the DVE quant rate ahead of PE consumption
                        nc.scalar.copy(wq8[g][c][:, 1, :], dst)

                def quant_group(c, kg, wst):
                    for kk in range(KG):
                        quant_one(c, kg * KG + kk, wst[:, kk, :])

                # column 0 W DMAs interleaved with tb0/tb1 x blocks.
                xp0 = xp1 = None
                w0 = []
                for kg in range(N_KG):
                    w0.append(load_w_group(0, kg))
                    if kg == 0:
                        xp0 = load_x_block(0)
                    elif kg == 1 and T_BLOCKS >= 2:
                        xp1 = load_x_block(1)
                for kg in range(N_KG):
                    quant_group(0, kg, w0[kg])
                for c in range(1, N_CHUNKS):
                    tiles = [load_w_group(c, kg) for kg in range(N_KG)]
                    for kg in range(N_KG):
                        quant_group(c, kg, tiles[kg])

            # --- main loop: uniform DoubleRow matmuls + fused drain ---
            ot_cur = {}  # mg -> (tile, tb)

            def flush_ot(mg):
                if mg in ot_cur:
                    t, tb_prev = ot_cur.pop(mg)
                    nc.scalar.dma_start(
                        out_r[:, ts(mg, MG), ts(tb_prev, TB)], t[:]
                    )

            def mm_tile(tb, m, xpt, flush_each=False):
                c, mi = divmod(m, M_PER_CHUNK)
                ps = pspool.tile([P, TB], F32, tag="ps")
                for g in range(N_MM):
                    nc.tensor.matmul(
                        ps[:],
                        wq8[g][c][:, :, ts(mi, P)],
                        xpt[:, g, :, :],
                        start=(g == 0),
                        stop=(g == N_MM - 1),
                        perf_mode=DR,
                    )
                mg, mgi = divmod(m, MG)
                if mgi == 0:
                    flush_ot(mg)
                    ot_tile = opool.tile(
                        [P, MG, TB], F32, tag=f"ot{mg % 2}", name=f"ot{mg % 2}"
                    )
                    ot_cur[mg] = (ot_tile, tb)
                ot, _ = ot_cur[mg]
                nc.scalar.activation(
                    ot[:, mgi, :],
                    ps[:],
                    mybir.ActivationFunctionType.Identity,
                    bias=bs[:, m : m + 1],
                    scale=scl_t[:, 0:1],
                )
                if flush_each:
                    nc.scalar.dma_start(out_r[:, m, ts(tb, TB)], ot[:, mgi, :])
                    if mgi == MG - 1:
                        ot_cur.pop(mg)
                elif mgi == MG - 1:
                    flush_ot(mg)

            if xp1 is not None and N_CHUNKS >= 2:
                # Software-pipeline tb0/tb1: alternate weight chunks between
                # the two blocks so quantization stays ahead of the PE.
                for c in range(N_CHUNKS):
                    for m in range(c * M_PER_CHUNK, (c + 1) * M_PER_CHUNK):
                        mm_tile(0, m, xp0)
                    for m in range(c * M_PER_CHUNK, (c + 1) * M_PER_CHUNK):
                        mm_tile(1, m, xp1)
                done = 2
            elif xp1 is not None:
                for m in range(M_TILES):
                    mm_tile(0, m, xp0)
                for m in range(M_TILES):
                    mm_tile(1, m, xp1)
                done = 2
            else:
                for m in range(M_TILES):
                    mm_tile(0, m, xp0)
                done = 1

            for tb in range(done, T_BLOCKS):
                xpt = load_x_block(tb)
                for m in range(M_TILES):
                    last_group = tb == T_BLOCKS - 1 and m >= M_TILES - MG
                    mm_tile(tb, m, xpt, flush_each=last_group)
            for mg in list(ot_cur):
                flush_ot(mg)

    nc.compile()
    return nc


def _host_beta_cut(W: np.ndarray):
    """beta exactly as the (jax) reference computes it, plus the exact fp32
    threshold c reproducing round-half-to-even of W/beta near 0.5."""
    try:
        import jax
        import jax.numpy as jnp

        cpu = jax.local_devices(backend="cpu")[0]
        with jax.default_device(cpu):
            beta = np.float32(jnp.maximum(jnp.mean(jnp.abs(jnp.asarray(W))), EPS))
    except Exception:
        beta = np.float32(max(np.abs(W).astype(np.float64).mean(), EPS))

    v = np.float32(0.5) * beta  # exact (power-of-two scale)
    assert np.float32(v / beta) <= np.float32(0.5)
    while True:
        nv = np.nextafter(v, np.float32(np.inf))
        if np.float32(nv / beta) <= np.float32(0.5):
            v = nv
        else:
            break
    return beta, v


def _pack_x(blk_T: np.ndarray, TB: int = 512) -> np.ndarray:
    """blk_T: [KI, TC] f32 -> packed [P, T_BLOCKS, N_MM, 2, TB] f8e4 with
    per-partition-contiguous token blocks (single-descriptor DMAs)."""
    KI, TC = blk_T.shape
    kb = C_COR * P
    hi = blk_T.astype(ml_dtypes.float8_e4m3fn)
    lo = (blk_T[:kb] - hi[:kb].astype(np.float32)).astype(
        ml_dtypes.float8_e4m3fn
    )
    xpair = np.empty((N_MM, 2, P, TC), dtype=ml_dtypes.float8_e4m3fn)
    xpair[:C_COR, 0] = hi[:kb].reshape(C_COR, P, TC)
    xpair[:C_COR, 1] = lo.reshape(C_COR, P, TC)
    xpair[C_COR:] = hi[kb:].reshape(N_MM - C_COR, 2, P, TC)
    # [g, s, p, (tb tbi)] -> [p, tb, g, s, tbi]
    v = xpair.reshape(N_MM, 2, P, TC // TB, TB)
    return np.ascontiguousarray(v.transpose(2, 3, 0, 1, 4))


def _pack_w(wT: np.ndarray, KG: int = 4, CHUNK: int = 512) -> np.ndarray:
    """wT: [KI, OC] f32 -> [P, N_CHUNKS, N_KG, KG, CHUNK] staging layout."""
    KI, OC = wT.shape
    n_kg = KI // P // KG
    n_ch = OC // CHUNK
    v = wT.reshape(n_kg, KG, P, n_ch, CHUNK)
    return np.ascontiguousarray(v.transpose(2, 3, 0, 1, 4))


def kernel(x: np.ndarray, W: np.ndarray, b: np.ndarray) -> np.ndarray:
    out, _ = _run(x, W, b)
    return out


def _run(x, W, b, **spmd_kwargs):
    x = np.ascontiguousarray(np.asarray(x, dtype=np.float32))
    W = np.ascontiguousarray(np.asarray(W, dtype=np.float32))
    b = np.ascontiguousarray(np.asarray(b, dtype=np.float32))

    B, T, KI = x.shape
    OC_full, KI2 = W.shape
    assert KI == KI2 == IN_FEATURES and OC_full == OUT_FEATURES
    NT = B * T
    assert NT == N_TOKENS

    TC = NT // S_WAYS
    OC = OUT_FEATURES // Q_WAYS

    beta, c = _host_beta_cut(W)
    S = np.float32(beta * LAYER_SCALE)
    consts_a = np.ascontiguousarray(
        np.broadcast_to(
            np.array([c, np.float32(-c), S], dtype=np.float32), (P, 3)
        )
    )

    xf = x.reshape(NT, KI)
    xp_s = [
        _pack_x(np.ascontiguousarray(xf[s * TC : (s + 1) * TC, :].T))
        for s in range(S_WAYS)
    ]
    wt_q = [
        _pack_w(np.ascontiguousarray(W[q * OC : (q + 1) * OC, :].T))
        for q in range(Q_WAYS)
    ]
    m_tiles = OC // P
    b_q = [
        np.ascontiguousarray(
            b[q * OC : (q + 1) * OC].reshape(m_tiles, P).T.ravel()
        )
        for q in range(Q_WAYS)
    ]

    in_maps = []
    for s in range(S_WAYS):
        for q in range(Q_WAYS):
            in_maps.append(
                {
                    "xp": xp_s[s],
                    "wt": wt_q[q],
                    "bvec": b_q[q],
                    "consts": consts_a,
                }
            )

    nc = build_nc(KI, OC, TC)
    res = run_bass_kernel_spmd(nc, in_maps, core_ids=list(range(N_CORES)), **spmd_kwargs)

    out_full = np.empty((NT, OUT_FEATURES), dtype=np.float32)
    for s in range(S_WAYS):
        for q in range(Q_WAYS):
            piece = res.results[s * Q_WAYS + q]["out"]  # [OC, TC]
            out_full[s * TC : (s + 1) * TC, q * OC : (q + 1) * OC] = piece.T
    return out_full.reshape(B, T, OUT_FEATURES), res


# revision 3
# speedup vs baseline: 2.4538x; 1.0021x over previous
"""BitLinear (ternary weight quantization + linear) on 8 Trainium2 NeuronCores.

Math: out = (x @ w_q.T + b) * LAYER_SCALE, where
  beta = max(mean(|W|), eps)           (global scalar over the full W)
  w_q  = clip(round(W / beta), -1, 1) * beta   (ternary: beta * {-1, 0, +1})

Device strategy (column-parallel + data-parallel as the baseline):
  8 cores = 2 batch-shards (tokens) x 4 feature-shards (out_features).

All-fp8 DoubleRow contraction. Every matmul is an e4m3 DoubleRow MM
(K=256 slots per instruction, issued back-to-back at the same ~216ns rate
as a K=128 bf16 MM — measured; mixing bf16 and fp8 MMs instead slows the
whole PE stream by exactly 1.2x, so the kernel stays dtype-pure).

Precision plan (slot assignment per output tile, 13 MMs x 256 slots):
  - k-subtiles 0..C_COR-1 ("corrected"): one MM per subtile whose two slots
    carry (hi, lo) = (e4m3(x), e4m3(x - hi)) against the SAME ternary weight
    in both slots. hi+lo reconstructs x to ~7 significant bits: error
    contribution ~7e-4.
  - k-subtiles C_COR..15: pairs of subtiles share one MM (slots = the two
    subtiles' e4m3(x)): full 2x rate, e4m3 rounding error only.
  With C_COR=10: 13 MMs/tile (vs 16 bf16 MMs in the baseline), measured
  end-to-end relative L2 error 1.63e-2 (gate 2e-2).

Ternary weights are exact in e4m3. beta and the exact |W| > c ternary
threshold are computed on host exactly as the baseline (bit-identical
quantization decisions).
"""

import math
from functools import lru_cache

import ml_dtypes
import numpy as np

import concourse.bass as bass
import concourse.mybir as mybir
import concourse.tile as tile
from concourse import bacc
from concourse.bass import ts
from concourse.bass_utils import run_bass_kernel_spmd

P = 128
IN_FEATURES = 2048
OUT_FEATURES = 8192
N_TOKENS = 8192  # 4 * 2048
EPS = 1e-8
LAYER_SCALE = np.float32(1.0 / math.sqrt(IN_FEATURES))

S_WAYS = 2  # data-parallel over tokens
Q_WAYS = 4  # tensor-parallel over out_features
N_CORES = S_WAYS * Q_WAYS

K_TILES = IN_FEATURES // P       # 16 k-subtiles
C_COR = 10                       # corrected k-subtiles (hi+lo pairs)
N_FAST = K_TILES - C_COR         # subtiles at plain e4m3 (paired 2-per-MM)
assert N_FAST % 2 == 0
N_MM = C_COR + N_FAST // 2       # DoubleRow MMs per output tile
KROWS = 2 * N_MM * P             # rows of the packed x input

F32 = mybir.dt.float32
F8E4 = mybir.dt.float8e4
DR = mybir.MatmulPerfMode.DoubleRow


@lru_cache(maxsize=4)
def build_nc(KI: int, OC: int, TC: int, TB: int = 512):
    """Per-core bass program.

    Inputs (per core; xp/wt are host-relaid so every DMA is one contiguous
    descriptor per partition — DMA issue time, not bandwidth, gated startup):
      xp     [P, T_BLOCKS, N_MM, 2, TB] f8e4: packed x slots; [p, tb, g, s, t]
             = slot s of MM g (hi/lo for g<C_COR, subtile-pair hi otherwise)
      wt     [P, N_CHUNKS, N_KG, KG, CHUNK] f32: W^T shard, chunk-major
             per-partition-contiguous staging groups
      bvec   [OC]     f32 : bias shard, host-reordered
      consts [P, 3]   f32 : [c, -c, beta*LAYER_SCALE] per partition
    Output:
      out  [OC, TC] f32 : (x @ w_q.T)^T shard, scaled and biased
    """
    assert KI % P == 0 and OC % P == 0 and TC % TB == 0
    assert KI // P == K_TILES
    M_TILES = OC // P
    T_BLOCKS = TC // TB
    KG = 4                     # k-tiles per W staging DMA
    N_KG = K_TILES // KG
    MG = min(4, M_TILES)       # m-tiles per output DMA
    assert K_TILES % KG == 0 and M_TILES % MG == 0
    CHUNK = min(512, OC)
    N_CHUNKS = OC // CHUNK

    nc = bacc.Bacc(None, target_bir_lowering=False, name="bitlinear")

    xp = nc.dram_tensor("xp", [P, T_BLOCKS, N_MM, 2, TB], F8E4,
                        kind="ExternalInput")
    wt = nc.dram_tensor("wt", [P, N_CHUNKS, N_KG, KG, CHUNK], F32,
                        kind="ExternalInput")
    bvec = nc.dram_tensor("bvec", [OC], F32, kind="ExternalInput")
    consts = nc.dram_tensor("consts", [P, 3], F32, kind="ExternalInput")
    out = nc.dram_tensor("out", [OC, TC], F32, kind="ExternalOutput")

    out_r = out[:].rearrange("(g p) t -> p g t", p=P)         # [P, M_TILES, TC]

    with tile.TileContext(nc) as tc:
        with (
            tc.tile_pool(name="const", bufs=1) as cpool,
            tc.tile_pool(name="wq", bufs=1) as wqpool,
            tc.tile_pool(name="xb", bufs=3) as xbpool,
            tc.tile_pool(name="ot", bufs=3) as opool,
            tc.tile_pool(name="ps", bufs=8, space="PSUM") as pspool,
        ):
            # --- constants ---
            cst = cpool.tile([P, 3], F32)
            bt = cpool.tile([P, M_TILES], F32)
            bs = cpool.tile([P, M_TILES], F32)
            nc.sync.dma_start(cst[:], consts[:])
            cut_t = cst[:, 0:1]
            ncut_t = cst[:, 1:2]
            scl_t = cst[:, 2:3]
            nc.sync.dma_start(bt[:], bvec[:].rearrange("(p m) -> p m", p=P))
            nc.gpsimd.tensor_scalar_mul(bs[:], bt[:], float(LAYER_SCALE))

            def load_x_block(tb):
                xt = xbpool.tile([P, N_MM, 2, TB], F8E4, tag="xpb", name="xpb")
                nc.sync.dma_start(xt[:], xp[:, tb, :, :, :])
                return xt

            # --- quantize W shard into e4m3 DoubleRow pair tiles.
            # wq8[g][c]: [P, 2, CHUNK]; g<C_COR: both slots = subtile g's
            # ternary weights (slot1 copied); g>=C_COR: slot s = subtile
            # C_COR + 2*(g-C_COR) + s. Chunk-major production order. ---
            M_PER_CHUNK = CHUNK // P
            wq8 = [[None] * N_CHUNKS for _ in range(N_MM)]
            with (
                tc.tile_pool(name="wstage", bufs=4) as wspool,
                tc.tile_pool(name="qtmp", bufs=6) as qpool,
            ):

                def load_w_group(c, kg):
                    wst = wspool.tile([P, KG, CHUNK], F32, tag="wst")
                    nc.sync.dma_start(wst[:], wt[:, c, kg, :, :])
                    return wst

                def quant_one(c, k, wst_slice):
                    if k < C_COR:
                        g, s = k, 0
                    else:
                        g = C_COR + (k - C_COR) // 2
                        s = (k - C_COR) % 2
                    if s == 0 and wq8[g][c] is None:
                        wq8[g][c] = wqpool.tile(
                            [P, 2, CHUNK], F8E4, tag=f"w8{g}_{c}",
                            name=f"w8{g}_{c}"
                        )
                    dst = wq8[g][c][:, s, :]
                    neg = qpool.tile([P, CHUNK], F32, tag="neg", name="neg")
                    nc.vector.tensor_scalar(
                        neg[:],
                        wst_slice,
                        ncut_t[:, 0:1],
                        None,
                        mybir.AluOpType.is_lt,
                    )
                    # wq = (W > c) - (W < -c)
                    nc.vector.scalar_tensor_tensor(
                        dst,
                        wst_slice,
                        cut_t[:, 0:1],
                        neg[:],
                        mybir.AluOpType.is_gt,
                        mybir.AluOpType.subtract,
                    )
                    if k < C_COR:
                        # duplicate ternary weights into slot 1 (same k for
                        # both hi and lo x slots) — on the Scalar engine, to
                        # keep the DVE quant rate ahead of PE consumption
                        nc.scalar.copy(wq8[g][c][:, 1, :], dst)

                def quant_group(c, kg, wst):
                    for kk in range(KG):
                        quant_one(c, kg * KG + kk, wst[:, kk, :])

                # column 0 W DMAs interleaved with tb0/tb1 x blocks.
                xp0 = xp1 = None
                w0 = []
                for kg in range(N_KG):
                    w0.append(load_w_group(0, kg))
                    if kg == 0:
                        xp0 = load_x_block(0)
                    elif kg == 1 and T_BLOCKS >= 2:
                        xp1 = load_x_block(1)
                for kg in range(N_KG):
                    quant_group(0, kg, w0[kg])
                for c in range(1, N_CHUNKS):
                    tiles = [load_w_group(c, kg) for kg in range(N_KG)]
                    for kg in range(N_KG):
                        quant_group(c, kg, tiles[kg])

            # --- main loop: uniform DoubleRow matmuls + fused drain ---
            ot_cur = {}  # mg -> (tile, tb)

            def flush_ot(mg):
                if mg in ot_cur:
                    t, tb_prev = ot_cur.pop(mg)
                    # stripe output flushes across both DMA-issue queues so
                    # neither ring saturates (~93 GB/s per ring; the full
                    # output stream is ~32 MiB)
                    eng = nc.scalar if mg % 2 == 0 else nc.sync
                    eng.dma_start(
                        out_r[:, ts(mg, MG), ts(tb_prev, TB)], t[:]
                    )

            def mm_tile(tb, m, xpt, flush_each=False):
                c, mi = divmod(m, M_PER_CHUNK)
                ps = pspool.tile([P, TB], F32, tag="ps")
                for g in range(N_MM):
                    nc.tensor.matmul(
                        ps[:],
                        wq8[g][c][:, :, ts(mi, P)],
                        xpt[:, g, :, :],
                        start=(g == 0),
                        stop=(g == N_MM - 1),
                        perf_mode=DR,
                    )
                mg, mgi = divmod(m, MG)
                if mgi == 0:
                    flush_ot(mg)
                    ot_tile = opool.tile(
                        [P, MG, TB], F32, tag=f"ot{mg % 2}", name=f"ot{mg % 2}"
                    )
                    ot_cur[mg] = (ot_tile, tb)
                ot, _ = ot_cur[mg]
                nc.scalar.activation(
                    ot[:, mgi, :],
                    ps[:],
                    mybir.ActivationFunctionType.Identity,
                    bias=bs[:, m : m + 1],
                    scale=scl_t[:, 0:1],
                )
                if flush_each:
                    eng = nc.scalar if m % 2 == 0 else nc.sync
                    eng.dma_start(out_r[:, m, ts(tb, TB)], ot[:, mgi, :])
                    if mgi == MG - 1:
                        ot_cur.pop(mg)
                elif mgi == MG - 1:
                    flush_ot(mg)

            if xp1 is not None and N_CHUNKS >= 2:
                # Software-pipeline tb0/tb1: alternate weight chunks between
                # the two blocks so quantization stays ahead of the PE.
                for c in range(N_CHUNKS):
                    for m in range(c * M_PER_CHUNK, (c + 1) * M_PER_CHUNK):
                        mm_tile(0, m, xp0)
                    for m in range(c * M_PER_CHUNK, (c + 1) * M_PER_CHUNK):
                        mm_tile(1, m, xp1)
                done = 2
            elif xp1 is not None:
                for m in range(M_TILES):
                    mm_tile(0, m, xp0)
                for m in range(M_TILES):
                    mm_tile(1, m, xp1)
                done = 2
            else:
                for m in range(M_TILES):
                    mm_tile(0, m, xp0)
                done = 1

            for tb in range(done, T_BLOCKS):
                xpt = load_x_block(tb)
                for m in range(M_TILES):
                    last_group = tb == T_BLOCKS - 1 and m >= M_TILES - MG
                    mm_tile(tb, m, xpt, flush_each=last_group)
            for mg in list(ot_cur):
                flush_ot(mg)

    nc.compile()
    return nc


def _host_beta_cut(W: np.ndarray):
    """beta exactly as the (jax) reference computes it, plus the exact fp32
    threshold c reproducing round-half-to-even of W/beta near 0.5."""
    try:
        import jax
        import jax.numpy as jnp

        cpu = jax.local_devices(backend="cpu")[0]
        with jax.default_device(cpu):
            beta = np.float32(jnp.maximum(jnp.mean(jnp.abs(jnp.asarray(W))), EPS))
    except Exception:
        beta = np.float32(max(np.abs(W).astype(np.float64).mean(), EPS))

    v = np.float32(0.5) * beta  # exact (power-of-two scale)
    assert np.float32(v / beta) <= np.float32(0.5)
    while True:
        nv = np.nextafter(v, np.float32(np.inf))
        if np.float32(nv / beta) <= np.float32(0.5):
            v = nv
        else:
            break
    return beta, v


def _pack_x(blk_T: np.ndarray, TB: int = 512) -> np.ndarray:
    """blk_T: [KI, TC] f32 -> packed [P, T_BLOCKS, N_MM, 2, TB] f8e4 with
    per-partition-contiguous token blocks (single-descriptor DMAs)."""
    KI, TC = blk_T.shape
    kb = C_COR * P
    hi = blk_T.astype(ml_dtypes.float8_e4m3fn)
    lo = (blk_T[:kb] - hi[:kb].astype(np.float32)).astype(
        ml_dtypes.float8_e4m3fn
    )
    xpair = np.empty((N_MM, 2, P, TC), dtype=ml_dtypes.float8_e4m3fn)
    xpair[:C_COR, 0] = hi[:kb].reshape(C_COR, P, TC)
    xpair[:C_COR, 1] = lo.reshape(C_COR, P, TC)
    xpair[C_COR:] = hi[kb:].reshape(N_MM - C_COR, 2, P, TC)
    # [g, s, p, (tb tbi)] -> [p, tb, g, s, tbi]
    v = xpair.reshape(N_MM, 2, P, TC // TB, TB)
    return np.ascontiguousarray(v.transpose(2, 3, 0, 1, 4))


def _pack_w(wT: np.ndarray, KG: int = 4, CHUNK: int = 512) -> np.ndarray:
    """wT: [KI, OC] f32 -> [P, N_CHUNKS, N_KG, KG, CHUNK] staging layout."""
    KI, OC = wT.shape
    n_kg = KI // P // KG
    n_ch = OC // CHUNK
    v = wT.reshape(n_kg, KG, P, n_ch, CHUNK)
    return np.ascontiguousarray(v.transpose(2, 3, 0, 1, 4))


def kernel(x: np.ndarray, W: np.ndarray, b: np.ndarray) -> np.ndarray:
    out, _ = _run(x, W, b)
    return out


def _run(x, W, b, **spmd_kwargs):
    x = np.ascontiguousarray(np.asarray(x, dtype=np.float32))
    W = np.ascontiguousarray(np.asarray(W, dtype=np.float32))
    b = np.ascontiguousarray(np.asarray(b, dtype=np.float32))

    B, T, KI = x.shape
    OC_full, KI2 = W.shape
    assert KI == KI2 == IN_FEATURES and OC_full == OUT_FEATURES
    NT = B * T
    assert NT == N_TOKENS

    TC = NT // S_WAYS
    OC = OUT_FEATURES // Q_WAYS

    beta, c = _host_beta_cut(W)
    S = np.float32(beta * LAYER_SCALE)
    consts_a = np.ascontiguousarray(
        np.broadcast_to(
            np.array([c, np.float32(-c), S], dtype=np.float32), (P, 3)
        )
    )

    xf = x.reshape(NT, KI)
    xp_s = [
        _pack_x(np.ascontiguousarray(xf[s * TC : (s + 1) * TC, :].T))
        for s in range(S_WAYS)
    ]
    wt_q = [
        _pack_w(np.ascontiguousarray(W[q * OC : (q + 1) * OC, :].T))
        for q in range(Q_WAYS)
    ]
    m_tiles = OC // P
    b_q = [
        np.ascontiguousarray(
            b[q * OC : (q + 1) * OC].reshape(m_tiles, P).T.ravel()
        )
        for q in range(Q_WAYS)
    ]

    in_maps = []
    for s in range(S_WAYS):
        for q in range(Q_WAYS):
            in_maps.append(
                {
                    "xp": xp_s[s],
                    "wt": wt_q[q],
                    "bvec": b_q[q],
                    "consts": consts_a,
                }
            )

    nc = build_nc(KI, OC, TC)
    res = run_bass_kernel_spmd(nc, in_maps, core_ids=list(range(N_CORES)), **spmd_kwargs)

    out_full = np.empty((NT, OUT_FEATURES), dtype=np.float32)
    for s in range(S_WAYS):
        for q in range(Q_WAYS):
            piece = res.results[s * Q_WAYS + q]["out"]  # [OC, TC]
            out_full[s * TC : (s + 1) * TC, q * OC : (q + 1) * OC] = piece.T
    return out_full.reshape(B, T, OUT_FEATURES), res
